# revision 9
# baseline (speedup 1.0000x reference)
"""Causal self-attention with RoPE for TRN2, sharded over 8 NeuronCores.

Sharding: data-parallel over batch (b=2) x tensor-parallel over head groups
(16 heads -> 4 groups of 4). Core c handles batch c//4, heads 4*(c%4)..4*(c%4)+3.
Each core projects q/k/v for its heads only, runs causal attention, then the
output projection is column-sharded: an AllGather of the per-group attention
outputs (within each batch's 4-core group) lets every core compute a distinct
512-wide column slice of the output. No all-reduce needed.

Storage is bf16 (x, weights, q/k/v, probs, attention outputs); all matmul
accumulation is fp32 in PSUM. Softmax denominators accumulate in
float32r. Single pass over x; q/k/v stay resident in SBUF (no DRAM spills).

Layout notes (per core):
  xT       [d_model, s]  = x[b].T
  wqT/wkT  [d_model, 512] rows of wq/wk for this head group, transposed;
           wq pre-scaled by 1/sqrt(d_k); rows permuted even-first within each
           head so RoPE pairs become contiguous partition halves.
  q/k      [d_k, s] per head (o on partitions) -> scores^T matmul directly
  v        [s-tiles, oc] (s on partitions) -> PV matmul lhsT directly
  scores^T [m, n] tiles; softmax denominator via ones-matmul over an
           exp-accumulator; no max-subtraction (scores are O(1) by
           construction, exp is safe in fp32).
"""

import math
import numpy as np

D_MODEL = 2048
N_HEADS = 16
D_K = 128
THETA = 10000.0
B = 2
S = 2048
P = 128
H = 4          # heads per core
OC = 512       # output columns per core (= H * D_K)
N_CORES = 8

_CACHE = {}


def _build_nc(s=S):
    from contextlib import ExitStack
    import concourse.tile as tile
    import concourse.mybir as mybir
    from concourse import bacc

    f32 = mybir.dt.float32
    f32r = mybir.dt.float32r
    bf16 = mybir.dt.bfloat16
    NB = s // 512          # 512-wide blocks along sequence
    NT = s // P            # 128-wide tiles along sequence
    IT = D_MODEL // P      # 128-wide tiles along d_model

    nc = bacc.Bacc("TRN2", target_bir_lowering=False, debug=False,
                   num_devices=N_CORES)

    xT = nc.dram_tensor("xT", [D_MODEL, s], bf16, kind="ExternalInput")
    wqT = nc.dram_tensor("wqT", [D_MODEL, OC], bf16, kind="ExternalInput")
    wkT = nc.dram_tensor("wkT", [D_MODEL, OC], bf16, kind="ExternalInput")
    wvT = nc.dram_tensor("wvT", [D_MODEL, OC], bf16, kind="ExternalInput")
    woT = nc.dram_tensor("woT", [D_MODEL, OC], bf16, kind="ExternalInput")
    cosF = nc.dram_tensor("cosF", [P, s], f32, kind="ExternalInput")
    sinF = nc.dram_tensor("sinF", [P, s], f32, kind="ExternalInput")
    out = nc.dram_tensor("out", [s, OC], f32, kind="ExternalOutput")

    # causal masks for the 4 diagonal m-tiles of each 512-wide n-block:
    # mask_tt[p, nn] = 1 if (p + 128*tt) <= nn
    mask_np = [
        (np.arange(P)[:, None] + P * tt <= np.arange(512)[None, :]).astype(np.float32)
        for tt in range(4)
    ]
    mask_h = [nc.inline_tensor(mask_np[tt], name=f"mask{tt}") for tt in range(4)]

    with tile.TileContext(nc, trace_sim=False) as tc:
        with ExitStack() as ctx:
            wp = ctx.enter_context(tc.tile_pool(name="wp", bufs=3))
            xp = ctx.enter_context(tc.tile_pool(name="xp", bufs=2))
            qkv = ctx.enter_context(tc.tile_pool(name="qkv", bufs=4))
            sm = ctx.enter_context(tc.tile_pool(name="sm", bufs=2))
            const = ctx.enter_context(tc.tile_pool(name="const", bufs=1))
            psmm = ctx.enter_context(tc.tile_pool(name="psmm", bufs=2, space="PSUM"))
            pssc = ctx.enter_context(tc.tile_pool(name="pssc", bufs=2, space="PSUM"))
            psat = ctx.enter_context(tc.tile_pool(name="psat", bufs=2, space="PSUM"))
            psd = ctx.enter_context(tc.tile_pool(name="psd", bufs=2, space="PSUM"))
            dram = ctx.enter_context(tc.tile_pool(name="dram", bufs=1, space="DRAM"))

            attn_part = [
                dram.tile([OC, 512], bf16, tag=f"ap{j}", name=f"attn_part{j}")
                for j in range(NB)
            ]
            attn_full = [
                dram.tile([4 * OC, 512], bf16, tag=f"af{j}", name=f"attn_full{j}")
                for j in range(NB)
            ]

            # persistent consts
            masks = []
            for tt in range(4):
                mt = const.tile([P, 512], f32, tag=f"mask{tt}", name=f"maskt{tt}")
                nc.sync.dma_start(mt[:], mask_h[tt][:, :])
                masks.append(mt)
            ones = const.tile([P, 1], f32r, tag="ones", name="ones")
            nc.vector.memset(ones[:].bitcast(f32), 1.0)

            # ---------------- Phase 1: q/k/v projections, single x pass -----
            wq_sb = wp.tile([P, IT, 512], bf16, tag="w", name="wq_sb")
            wk_sb = wp.tile([P, IT, 512], bf16, tag="w", name="wk_sb")
            wv_sb = wp.tile([P, IT, 512], bf16, tag="w", name="wv_sb")
            for w_sb, w_dr in ((wq_sb, wqT), (wk_sb, wkT), (wv_sb, wvT)):
                for i in range(IT):
                    nc.sync.dma_start(w_sb[:, i, :], w_dr[P * i:P * (i + 1), :])

            q_sb = qkv.tile([P, H, s], bf16, tag="qkv", name="q_sb")
            k_sb = qkv.tile([P, H, s], bf16, tag="qkv", name="k_sb")
            v_sb = qkv.tile([P, NT, 512], bf16, tag="qkv", name="v_sb")

            for b in range(NB):
                blk = slice(512 * b, 512 * (b + 1))
                xblk = xp.tile([P, IT, 512], bf16, tag="x", name=f"xb{b}")
                for i in range(IT):
                    nc.sync.dma_start(
                        xblk[:, i, :], xT[P * i:P * (i + 1), blk])
                ct = sm.tile([P, 512], f32, tag="ct", name=f"ct{b}")
                st = sm.tile([P, 512], f32, tag="st", name=f"st{b}")
                nc.sync.dma_start(ct[:], cosF[:, blk])
                nc.sync.dma_start(st[:], sinF[:, blk])

                # q and k projections with fused RoPE
                for w_sb, dst in ((wq_sb, q_sb), (wk_sb, k_sb)):
                    for ot in range(H):
                        ps = psmm.tile([P, 512], f32, tag="psmm",
                                       name=f"psqk{b}{ot}")
                        for i in range(IT):
                            nc.tensor.matmul(
                                ps[:],
                                w_sb[:, i, P * ot:P * (ot + 1)],
                                xblk[:, i, :],
                                start=(i == 0), stop=(i == IT - 1))
                        raw = sm.tile([P, 512], f32, tag="raw", name=f"raw{b}{ot}")
                        nc.scalar.copy(raw[:], ps[:])
                        sw = sm.tile([P, 512], f32, tag="sw", name=f"sw{b}{ot}")
                        nc.sync.dma_start(sw[0:64, :], raw[64:128, :])
                        nc.sync.dma_start(sw[64:128, :], raw[0:64, :])
                        nc.vector.tensor_mul(sw[:], sw[:], st[:])
                        nc.vector.tensor_mul(raw[:], raw[:], ct[:])
                        nc.vector.tensor_add(dst[:, ot, blk], raw[:], sw[:])

                # v projection: [s-tile, oc]
                for tt in range(4):
                    ps = psmm.tile([P, 512], f32, tag="psmm", name=f"psv{b}{tt}")
                    for i in range(IT):
                        nc.tensor.matmul(
                            ps[:],
                            xblk[:, i, P * tt:P * (tt + 1)],
                            wv_sb[:, i, :],
                            start=(i == 0), stop=(i == IT - 1))
                    nc.scalar.copy(v_sb[:, 4 * b + tt, :], ps[:])

            # ---------------- Phase 2: attention, n-block outer -------------
            attn_sb = qkv.tile([P, H, s], bf16, tag="qkv", name="attn_sb")
            for j in range(NB):
                for h in range(H):
                    n_sl = slice(512 * j, 512 * (j + 1))
                    n_mt = 4 * (j + 1)
                    ps_at = psat.tile([P, 512], f32, tag="psat", name=f"psat{h}{j}")
                    acc = sm.tile([P, 512], f32r, tag="acc", name=f"acc{h}{j}")
                    for t in range(n_mt):
                        ps_s = pssc.tile([P, 512], f32, tag="pssc",
                                         name=f"pss{h}{j}{t}")
                        nc.tensor.matmul(
                            ps_s[:], k_sb[:, h, P * t:P * (t + 1)], q_sb[:, h, n_sl],
                            start=True, stop=True)
                        ex = sm.tile([P, 512], bf16, tag="ex", name=f"ex{h}{j}{t}",
                                     bufs=4)
                        nc.scalar.activation(ex[:], ps_s[:],
                                             mybir.ActivationFunctionType.Exp)
                        if t >= 4 * j:
                            nc.gpsimd.tensor_mul(ex[:], ex[:], masks[t - 4 * j][:])
                        nc.tensor.matmul(
                            ps_at[:], v_sb[:, t, P * h:P * (h + 1)], ex[:],
                            start=(t == 0), stop=(t == n_mt - 1))
                        if t == 0:
                            nc.vector.tensor_copy(acc[:], ex[:])
                        else:
                            nc.vector.tensor_add(acc[:], acc[:], ex[:])
                    # denominator + normalize
                    ps_D = psd.tile([1, 512], f32, tag="psd", name=f"psd{h}{j}")
                    nc.tensor.matmul(ps_D[:], ones[:], acc[:], start=True, stop=True)
                    rec = sm.tile([1, 512], f32, tag="rec", name=f"rec{h}{j}")
                    nc.vector.reciprocal(rec[:], ps_D[:])
                    rec_b = sm.tile([P, 512], f32, tag="recb", name=f"recb{h}{j}")
                    nc.gpsimd.partition_broadcast(rec_b[:], rec[:])
                    nc.vector.tensor_mul(attn_sb[:, h, n_sl], ps_at[:], rec_b[:])
                # ship this n-block: AllGather overlaps with later blocks
                for h in range(H):
                    nc.sync.dma_start(attn_part[j][P * h:P * (h + 1), :],
                                      attn_sb[:, h, n_sl])
                nc.gpsimd.collective_compute(
                    "AllGather",
                    mybir.AluOpType.bypass,
                    replica_groups=[[0, 1, 2, 3], [4, 5, 6, 7]],
                    ins=[attn_part[j][:].opt()],
                    outs=[attn_full[j][:].opt()],
                )

            # ---------------- Phase 3: output projection --------------------
            wo_sb = wp.tile([P, IT, 512], bf16, tag="w", name="wo_sb")
            for i in range(IT):
                nc.sync.dma_start(wo_sb[:, i, :], woT[P * i:P * (i + 1), :])
            for ss in range(NT):
                af = sm.tile([P, IT, P], bf16, tag="af", name=f"af{ss}")
                for jj in range(IT):
                    nc.sync.dma_start(
                        af[:, jj, :],
                        attn_full[ss // 4][P * jj:P * (jj + 1),
                                           P * (ss % 4):P * (ss % 4 + 1)])
                ps_o = psmm.tile([P, 512], f32, tag="psmm", name=f"pso{ss}")
                for jj in range(IT):
                    nc.tensor.matmul(ps_o[:], af[:, jj, :], wo_sb[:, jj, :],
                                     start=(jj == 0), stop=(jj == IT - 1))
                ob = sm.tile([P, 512], f32, tag="ob", name=f"ob{ss}")
                nc.scalar.copy(ob[:], ps_o[:])
                nc.sync.dma_start(out[P * ss:P * (ss + 1), :], ob[:])

    nc.compile()
    return nc


def _get_nc(s=S):
    if s not in _CACHE:
        _CACHE[s] = _build_nc(s)
    return _CACHE[s]


def _host_prep(x, wq, wk, wv, wo, token_positions, s=S):
    """Build per-core input maps."""
    import ml_dtypes
    bf = ml_dtypes.bfloat16

    # even-first permutation within each head (applied to wq, wk output rows)
    perm = np.concatenate([np.arange(0, D_K, 2), np.arange(1, D_K, 2)])
    full_perm = np.concatenate([h * D_K + perm for h in range(N_HEADS)])
    wq_p = (wq / math.sqrt(D_K))[full_perm, :]
    wk_p = wk[full_perm, :]

    # rope tables per batch: cosF/sinF [128, s]
    power = np.arange(0, D_K, 2, dtype=np.float32) / D_K
    freq = 1.0 / (THETA ** power)                      # [64]
    trig = []
    for b in range(B):
        ang = token_positions[b].astype(np.float32)[None, :] * freq[:, None]  # [64, s]
        c = np.cos(ang).astype(np.float32)
        sn = np.sin(ang).astype(np.float32)
        cosFv = np.concatenate([c, c], axis=0)          # [128, s]
        sinFv = np.concatenate([-sn, sn], axis=0)       # [128, s]
        trig.append((np.ascontiguousarray(cosFv), np.ascontiguousarray(sinFv)))

    in_maps = []
    for c in range(N_CORES):
        b, g = c // 4, c % 4
        rows = slice(OC * g, OC * (g + 1))
        in_maps.append({
            "xT": np.ascontiguousarray(x[b].T).astype(bf),
            "wqT": np.ascontiguousarray(wq_p[rows, :].T).astype(bf),
            "wkT": np.ascontiguousarray(wk_p[rows, :].T).astype(bf),
            "wvT": np.ascontiguousarray(wv[rows, :].T).astype(bf),
            "woT": np.ascontiguousarray(wo[rows, :].T).astype(bf),
            "cosF": trig[b][0],
            "sinF": trig[b][1],
        })
    return in_maps


def kernel(x, wq, wk, wv, wo, token_positions):
    from concourse.bass_utils import run_bass_kernel_spmd

    x = np.asarray(x, dtype=np.float32)
    wq = np.asarray(wq, dtype=np.float32)
    wk = np.asarray(wk, dtype=np.float32)
    wv = np.asarray(wv, dtype=np.float32)
    wo = np.asarray(wo, dtype=np.float32)
    token_positions = np.asarray(token_positions)

    s = x.shape[1]
    nc = _get_nc(s)
    in_maps = _host_prep(x, wq, wk, wv, wo, token_positions, s)
    res = run_bass_kernel_spmd(nc, in_maps, core_ids=list(range(N_CORES)))

    out = np.empty((B, s, D_MODEL), dtype=np.float32)
    for c in range(N_CORES):
        b, g = c // 4, c % 4
        out[b, :, OC * g:OC * (g + 1)] = res.results[c]["out"]
    return out


# revision 18
# speedup vs baseline: 1.1846x; 1.1846x over previous
"""Causal self-attention with RoPE for TRN2, sharded over 8 NeuronCores.

Sharding: data-parallel over batch (b=2) x tensor-parallel over head groups
(16 heads -> 4 groups of 4). Core c handles batch c//4, heads 4*(c%4)..4*(c%4)+3.
Each core projects q/k/v for its heads only, runs causal attention, then the
output projection is column-sharded: per-sequence-block AllGathers of the
per-group attention outputs (within each batch's 4-core group) let every core
compute a distinct 512-wide column slice of the output. No all-reduce needed.
The AllGathers are issued per n-block so they overlap with attention compute
of later blocks; output-projection blocks are interleaved one block behind.

Storage is bf16 (x, weights, q/k/v, probs, attention outputs); all matmul
accumulation is fp32 in PSUM. Softmax denominators accumulate in float32r.
Single pass over x; q/k/v stay resident in SBUF (no DRAM spills).

Layout notes (per core):
  xT       [d_model, s]  = x[b].T
  wqT/wkT  [d_model, 512] rows of wq/wk for this head group, transposed;
           wq pre-scaled by 1/sqrt(d_k); rows permuted even-first within each
           head so RoPE pairs become contiguous partition halves.
  q/k      [d_k, s] per head (o on partitions) -> scores^T matmul directly
  v        [s-tiles, oc] (s on partitions) -> PV matmul lhsT directly
  scores^T [m, n] tiles; softmax denominator via an all-ones [128,128] matmul
           over an exp-accumulator (yields D broadcast across partitions);
           no max-subtraction (scores are O(1) by construction, exp is safe
           in fp32). gpsimd runs ONLY the collectives so they never block
           compute engines.
"""

import math
import numpy as np

D_MODEL = 2048
N_HEADS = 16
D_K = 128
THETA = 10000.0
B = 2
S = 2048
P = 128
H = 4          # heads per core
OC = 512       # output columns per core (= H * D_K)
N_CORES = 8

_CACHE = {}


def _build_nc(s=S, reps=1):
    from contextlib import ExitStack
    import concourse.tile as tile
    import concourse.mybir as mybir
    from concourse import bacc

    f32 = mybir.dt.float32
    f32r = mybir.dt.float32r
    bf16 = mybir.dt.bfloat16
    NB = s // 512          # 512-wide blocks along sequence
    NT = s // P            # 128-wide tiles along sequence
    IT = D_MODEL // P      # 128-wide tiles along d_model

    nc = bacc.Bacc("TRN2", target_bir_lowering=False, debug=False,
                   num_devices=N_CORES)

    xT = nc.dram_tensor("xT", [D_MODEL, s], bf16, kind="ExternalInput")
    wqT = nc.dram_tensor("wqT", [D_MODEL, OC], bf16, kind="ExternalInput")
    wkT = nc.dram_tensor("wkT", [D_MODEL, OC], bf16, kind="ExternalInput")
    wvT = nc.dram_tensor("wvT", [D_MODEL, OC], bf16, kind="ExternalInput")
    woT = nc.dram_tensor("woT", [D_MODEL, OC], bf16, kind="ExternalInput")
    cosF = nc.dram_tensor("cosF", [P, s], f32, kind="ExternalInput")
    sinF = nc.dram_tensor("sinF", [P, s], f32, kind="ExternalInput")
    out = nc.dram_tensor("out", [s, OC], f32, kind="ExternalOutput")

    # triangle mask for the exactly-diagonal 128x128 tile: 1 if p <= c
    tri_np = (np.arange(P)[:, None] <= np.arange(P)[None, :]).astype(np.float32)
    tri_h = nc.inline_tensor(tri_np, name="tri_mask")

    with tile.TileContext(nc, trace_sim=False) as tc:
        with ExitStack() as ctx:
            wp = ctx.enter_context(tc.tile_pool(name="wp", bufs=3))
            xp = ctx.enter_context(tc.tile_pool(name="xp", bufs=2))
            qkv = ctx.enter_context(tc.tile_pool(name="qkv", bufs=4))
            sm = ctx.enter_context(tc.tile_pool(name="sm", bufs=2))
            const = ctx.enter_context(tc.tile_pool(name="const", bufs=1))
            psmm = ctx.enter_context(tc.tile_pool(name="psmm", bufs=2, space="PSUM"))
            pssc = ctx.enter_context(tc.tile_pool(name="pssc", bufs=3, space="PSUM"))
            psat = ctx.enter_context(tc.tile_pool(name="psat", bufs=2, space="PSUM"))
            psd = ctx.enter_context(tc.tile_pool(name="psd", bufs=1, space="PSUM"))
            dram = ctx.enter_context(tc.tile_pool(name="dram", bufs=1, space="DRAM"))

            attn_part = [
                dram.tile([OC, 512], bf16, tag=f"ap{j}", name=f"attn_part{j}")
                for j in range(NB)
            ]
            attn_full = [
                dram.tile([4 * OC, 512], bf16, tag=f"af{j}", name=f"attn_full{j}")
                for j in range(NB)
            ]

            # persistent consts
            tri = const.tile([P, P], f32, tag="tri", name="tri")
            nc.sync.dma_start(tri[:], tri_h[:, :])
            ones = const.tile([P, P], f32r, tag="ones", name="ones")
            nc.vector.memset(ones[:].bitcast(f32), 1.0)

            for _rep in range(reps):
                # ------------ Phase 1: q/k/v projections, single x pass -----
                wq_sb = wp.tile([P, IT, 512], bf16, tag="w", name="wq_sb")
                wk_sb = wp.tile([P, IT, 512], bf16, tag="w", name="wk_sb")
                wv_sb = wp.tile([P, IT, 512], bf16, tag="w", name="wv_sb")

                q_sb = qkv.tile([P, H, s], bf16, tag="qkv", name="q_sb")
                k_sb = qkv.tile([P, H, s], bf16, tag="qkv", name="k_sb")
                v_sb = qkv.tile([P, NT, 512], bf16, tag="qkv", name="v_sb")

                xblks = []
                for b in range(NB):
                    xblks.append(xp.tile([P, IT, 512], bf16, tag="x",
                                         name=f"xb{b}"))

                # first x block + trig first so PE starts ASAP, then weights
                # interleaved by i-tile (consumption order: q, k, then v)
                def load_xblk(b):
                    blk = slice(512 * b, 512 * (b + 1))
                    for i in range(IT):
                        nc.sync.dma_start(
                            xblks[b][:, i, :], xT[P * i:P * (i + 1), blk])

                load_xblk(0)
                for i in range(IT):
                    nc.sync.dma_start(wq_sb[:, i, :], wqT[P * i:P * (i + 1), :])
                    nc.sync.dma_start(wk_sb[:, i, :], wkT[P * i:P * (i + 1), :])
                    nc.sync.dma_start(wv_sb[:, i, :], wvT[P * i:P * (i + 1), :])

                def proj_block(b):
                    blk = slice(512 * b, 512 * (b + 1))
                    xblk = xblks[b]
                    if b + 1 < NB:
                        load_xblk(b + 1)  # prefetch next block
                    ct = sm.tile([P, 512], f32, tag="ct", name=f"ct{b}", bufs=2)
                    st = sm.tile([P, 512], f32, tag="st", name=f"st{b}", bufs=2)
                    nc.sync.dma_start(ct[:], cosF[:, blk])
                    nc.sync.dma_start(st[:], sinF[:, blk])

                    # q and k projections with fused RoPE
                    for w_sb, dst in ((wq_sb, q_sb), (wk_sb, k_sb)):
                        for ot in range(H):
                            ps = psmm.tile([P, 512], f32, tag="psmm",
                                           name=f"psqk{b}{ot}")
                            for i in range(IT):
                                nc.tensor.matmul(
                                    ps[:],
                                    w_sb[:, i, P * ot:P * (ot + 1)],
                                    xblk[:, i, :],
                                    start=(i == 0), stop=(i == IT - 1))
                            raw = sm.tile([P, 512], f32, tag="raw",
                                          name=f"raw{b}{ot}")
                            nc.scalar.copy(raw[:], ps[:])
                            sw = sm.tile([P, 512], f32, tag="sw", name=f"sw{b}{ot}")
                            nc.sync.dma_start(sw[0:64, :], raw[64:128, :])
                            nc.sync.dma_start(sw[64:128, :], raw[0:64, :])
                            nc.vector.tensor_mul(sw[:], sw[:], st[:])
                            nc.vector.tensor_mul(raw[:], raw[:], ct[:])
                            nc.vector.tensor_add(dst[:, ot, blk], raw[:], sw[:])

                    # v projection: [s-tile, oc]
                    for tt in range(4):
                        ps = psmm.tile([P, 512], f32, tag="psmm", name=f"psv{b}{tt}")
                        for i in range(IT):
                            nc.tensor.matmul(
                                ps[:],
                                xblk[:, i, P * tt:P * (tt + 1)],
                                wv_sb[:, i, :],
                                start=(i == 0), stop=(i == IT - 1))
                        nc.scalar.copy(v_sb[:, 4 * b + tt, :], ps[:])

                # ------------ Phase 2+3: attention / AG / outproj pipeline --
                attn_sb = qkv.tile([P, H, s], bf16, tag="qkv", name="attn_sb")

                def attn_block(j):
                    n_sl = slice(512 * j, 512 * (j + 1))
                    n_mt = 4 * (j + 1)
                    for h in range(H):
                        ps_at = psat.tile([P, 512], f32, tag="psat",
                                          name=f"psat{h}{j}")
                        acc = sm.tile([P, 512], f32r, tag="acc", name=f"acc{h}{j}")
                        for t in range(n_mt):
                            # diagonal tiles: only columns nn >= off are valid
                            off = max(0, P * (t - 4 * j))
                            w = 512 - off
                            ps_s = pssc.tile([P, 512], f32, tag="pssc",
                                             name=f"pss{h}{j}{t}")
                            nc.tensor.matmul(
                                ps_s[:, :w], k_sb[:, h, P * t:P * (t + 1)],
                                q_sb[:, h, 512 * j + off:512 * (j + 1)],
                                start=True, stop=True)
                            ex = sm.tile([P, 512], bf16, tag="ex",
                                         name=f"ex{h}{j}{t}", bufs=4)
                            nc.scalar.activation(ex[:, :w], ps_s[:, :w],
                                                 mybir.ActivationFunctionType.Exp)
                            if t >= 4 * j:
                                nc.vector.tensor_mul(ex[:, 0:P], ex[:, 0:P],
                                                     tri[:])
                            nc.tensor.matmul(
                                ps_at[:, off:512],
                                v_sb[:, t, P * h:P * (h + 1)], ex[:, :w],
                                start=(t == 0), stop=(t == n_mt - 1))
                            if t == 0:
                                nc.vector.tensor_copy(acc[:], ex[:])
                            else:
                                nc.vector.tensor_add(acc[:, off:512],
                                                     acc[:, off:512], ex[:, :w])
                        # denominator broadcast to all partitions via ones-matmul
                        ps_D = psd.tile([P, 512], f32, tag="psd", name=f"psd{h}{j}")
                        nc.tensor.matmul(ps_D[:], ones[:], acc[:],
                                         start=True, stop=True)
                        recb = sm.tile([P, 512], f32, tag="recb",
                                       name=f"recb{h}{j}")
                        nc.vector.reciprocal(recb[:], ps_D[:])
                        nc.vector.tensor_mul(attn_sb[:, h, n_sl], ps_at[:],
                                             recb[:])
                    # ship this n-block; collective runs on gpsimd/TOPSP only
                    for h in range(H):
                        nc.sync.dma_start(attn_part[j][P * h:P * (h + 1), :],
                                          attn_sb[:, h, n_sl])
                    nc.gpsimd.collective_compute(
                        "AllGather",
                        mybir.AluOpType.bypass,
                        replica_groups=[[0, 1, 2, 3], [4, 5, 6, 7]],
                        ins=[attn_part[j][:].opt()],
                        outs=[attn_full[j][:].opt()],
                    )

                def outproj_block(j):
                    for ss in range(4 * j, 4 * (j + 1)):
                        af = sm.tile([P, IT, P], bf16, tag="af", name=f"af{ss}",
                                     bufs=3)
                        nc.sync.dma_start(
                            af[:, :, :],
                            attn_full[j][:, P * (ss % 4):P * (ss % 4 + 1)]
                            .rearrange("(i p) c -> p i c", p=P))
                        ps_o = psmm.tile([P, 512], f32, tag="psmm",
                                         name=f"pso{ss}")
                        for jj in range(IT):
                            nc.tensor.matmul(ps_o[:], af[:, jj, :],
                                             wo_sb[:, jj, :],
                                             start=(jj == 0), stop=(jj == IT - 1))
                        ob = sm.tile([P, 512], f32, tag="ob", name=f"ob{ss}")
                        nc.scalar.copy(ob[:], ps_o[:])
                        nc.sync.dma_start(out[P * ss:P * (ss + 1), :], ob[:])

                # pipeline: project block b then run its attention (which
                # only needs k/v blocks <= b); AGs issue early and hide.
                # outproj afterwards -- PE never queues behind an AllGather.
                for b in range(NB):
                    proj_block(b)
                    attn_block(b)
                wo_sb = wp.tile([P, IT, 512], bf16, tag="w", name="wo_sb")
                for i in range(IT):
                    nc.sync.dma_start(wo_sb[:, i, :], woT[P * i:P * (i + 1), :])
                for j in range(NB):
                    outproj_block(j)

    nc.compile()
    return nc


def _get_nc(s=S, reps=1):
    key = (s, reps)
    if key not in _CACHE:
        _CACHE[key] = _build_nc(s, reps)
    return _CACHE[key]


def _host_prep(x, wq, wk, wv, wo, token_positions, s=S):
    """Build per-core input maps."""
    import ml_dtypes
    bf = ml_dtypes.bfloat16

    # even-first permutation within each head (applied to wq, wk output rows)
    perm = np.concatenate([np.arange(0, D_K, 2), np.arange(1, D_K, 2)])
    full_perm = np.concatenate([h * D_K + perm for h in range(N_HEADS)])
    wq_p = (wq / math.sqrt(D_K))[full_perm, :]
    wk_p = wk[full_perm, :]

    # rope tables per batch: cosF/sinF [128, s]
    power = np.arange(0, D_K, 2, dtype=np.float32) / D_K
    freq = 1.0 / (THETA ** power)                      # [64]
    trig = []
    for b in range(B):
        ang = token_positions[b].astype(np.float32)[None, :] * freq[:, None]  # [64, s]
        c = np.cos(ang).astype(np.float32)
        sn = np.sin(ang).astype(np.float32)
        cosFv = np.concatenate([c, c], axis=0)          # [128, s]
        sinFv = np.concatenate([-sn, sn], axis=0)       # [128, s]
        trig.append((np.ascontiguousarray(cosFv), np.ascontiguousarray(sinFv)))

    in_maps = []
    for c in range(N_CORES):
        b, g = c // 4, c % 4
        rows = slice(OC * g, OC * (g + 1))
        in_maps.append({
            "xT": np.ascontiguousarray(x[b].T).astype(bf),
            "wqT": np.ascontiguousarray(wq_p[rows, :].T).astype(bf),
            "wkT": np.ascontiguousarray(wk_p[rows, :].T).astype(bf),
            "wvT": np.ascontiguousarray(wv[rows, :].T).astype(bf),
            "woT": np.ascontiguousarray(wo[rows, :].T).astype(bf),
            "cosF": trig[b][0],
            "sinF": trig[b][1],
        })
    return in_maps


def kernel(x, wq, wk, wv, wo, token_positions):
    from concourse.bass_utils import run_bass_kernel_spmd

    x = np.asarray(x, dtype=np.float32)
    wq = np.asarray(wq, dtype=np.float32)
    wk = np.asarray(wk, dtype=np.float32)
    wv = np.asarray(wv, dtype=np.float32)
    wo = np.asarray(wo, dtype=np.float32)
    token_positions = np.asarray(token_positions)

    s = x.shape[1]
    nc = _get_nc(s)
    in_maps = _host_prep(x, wq, wk, wv, wo, token_positions, s)
    res = run_bass_kernel_spmd(nc, in_maps, core_ids=list(range(N_CORES)))

    out = np.empty((B, s, D_MODEL), dtype=np.float32)
    for c in range(N_CORES):
        b, g = c // 4, c % 4
        out[b, :, OC * g:OC * (g + 1)] = res.results[c]["out"]
    return out


# revision 23
# speedup vs baseline: 7.3198x; 6.1789x over previous
"""Causal self-attention with RoPE for TRN2, sharded over 8 NeuronCores.

Sharding: data-parallel over batch (b=2) x tensor-parallel over head groups
(16 heads -> 4 groups of 4). Core c handles batch c//4, heads 4*(c%4)..4*(c%4)+3.
Each core projects q/k/v for its heads only, runs causal attention, then the
output projection is column-sharded: per-sequence-block AllGathers of the
per-group attention outputs (within each batch's 4-core group) let every core
compute a distinct 512-wide column slice of the output. No all-reduce needed.
The AllGathers are issued per n-block so they overlap with attention compute
of later blocks; output-projection blocks are interleaved one block behind.

Storage is bf16 (x, weights, q/k/v, probs, attention outputs); all matmul
accumulation is fp32 in PSUM. Softmax denominators accumulate in float32r.
Single pass over x; q/k/v stay resident in SBUF (no DRAM spills).

Layout notes (per core):
  xT       [d_model, s]  = x[b].T
  wqT/wkT  [d_model, 512] rows of wq/wk for this head group, transposed;
           wq pre-scaled by 1/sqrt(d_k); rows permuted even-first within each
           head so RoPE pairs become contiguous partition halves.
  q/k      [d_k, s] per head (o on partitions) -> scores^T matmul directly
  v        [s-tiles, oc] (s on partitions) -> PV matmul lhsT directly
  scores^T [m, n] tiles; softmax denominator via an all-ones [128,128] matmul
           over an exp-accumulator (yields D broadcast across partitions);
           no max-subtraction (scores are O(1) by construction, exp is safe
           in fp32). gpsimd runs ONLY the collectives so they never block
           compute engines.
"""

import math
import numpy as np

D_MODEL = 2048
N_HEADS = 16
D_K = 128
THETA = 10000.0
B = 2
S = 2048
P = 128
H = 4          # heads per core
OC = 512       # output columns per core (= H * D_K)
N_CORES = 8

_CACHE = {}


def _build_nc(s=S, reps=1):
    from contextlib import ExitStack
    import concourse.tile as tile
    import concourse.mybir as mybir
    from concourse import bacc

    f32 = mybir.dt.float32
    f32r = mybir.dt.float32r
    bf16 = mybir.dt.bfloat16
    NB = s // 512          # 512-wide blocks along sequence
    NT = s // P            # 128-wide tiles along sequence
    IT = D_MODEL // P      # 128-wide tiles along d_model

    nc = bacc.Bacc("TRN2", target_bir_lowering=False, debug=False,
                   num_devices=N_CORES)

    xT = nc.dram_tensor("xT", [D_MODEL, s], bf16, kind="ExternalInput")
    wqT = nc.dram_tensor("wqT", [D_MODEL, OC], bf16, kind="ExternalInput")
    wkT = nc.dram_tensor("wkT", [D_MODEL, OC], bf16, kind="ExternalInput")
    wvT = nc.dram_tensor("wvT", [D_MODEL, OC], bf16, kind="ExternalInput")
    woT = nc.dram_tensor("woT", [D_MODEL, OC], bf16, kind="ExternalInput")
    cosF = nc.dram_tensor("cosF", [P, s], f32, kind="ExternalInput")
    sinF = nc.dram_tensor("sinF", [P, s], f32, kind="ExternalInput")
    out = nc.dram_tensor("out", [s, OC], f32, kind="ExternalOutput")

    # additive causal mask for the exactly-diagonal 128x128 tile:
    # scores[p, c] += -1e9 where p > c, applied in PSUM via a second matmul
    # (lhsT = -1e9 * I, rhs = strict-lower indicator), keeping masking on PE.
    import ml_dtypes
    negI_np = (np.eye(P) * -1e9).astype(ml_dtypes.bfloat16)
    tric_np = (np.arange(P)[:, None] < np.arange(P)[None, :]).astype(ml_dtypes.bfloat16).T
    negI_h = nc.inline_tensor(negI_np, name="negI")
    tric_h = nc.inline_tensor(tric_np, name="tric")

    with tile.TileContext(nc, trace_sim=False) as tc:
        with ExitStack() as ctx:
            wp = ctx.enter_context(tc.tile_pool(name="wp", bufs=3))
            xp = ctx.enter_context(tc.tile_pool(name="xp", bufs=2))
            qkv = ctx.enter_context(tc.tile_pool(name="qkv", bufs=4))
            sm = ctx.enter_context(tc.tile_pool(name="sm", bufs=2))
            const = ctx.enter_context(tc.tile_pool(name="const", bufs=1))
            psmm = ctx.enter_context(tc.tile_pool(name="psmm", bufs=2, space="PSUM"))
            pssc = ctx.enter_context(tc.tile_pool(name="pssc", bufs=3, space="PSUM"))
            psat = ctx.enter_context(tc.tile_pool(name="psat", bufs=2, space="PSUM"))
            psd = ctx.enter_context(tc.tile_pool(name="psd", bufs=1, space="PSUM"))
            dram = ctx.enter_context(tc.tile_pool(name="dram", bufs=1, space="DRAM"))

            attn_part = [
                dram.tile([OC, 512], bf16, tag=f"ap{j}", name=f"attn_part{j}")
                for j in range(NB)
            ]
            attn_full = [
                dram.tile([4 * OC, 512], bf16, tag=f"af{j}", name=f"attn_full{j}")
                for j in range(NB)
            ]

            # persistent consts
            negI = const.tile([P, P], bf16, tag="negI", name="negI")
            nc.sync.dma_start(negI[:], negI_h[:, :])
            tric = const.tile([P, P], bf16, tag="tric", name="tric")
            nc.sync.dma_start(tric[:], tric_h[:, :])
            ones = const.tile([P, P], f32r, tag="ones", name="ones")
            nc.vector.memset(ones[:].bitcast(f32), 1.0)

            for _rep in range(reps):
                # ------------ Phase 1: q/k/v projections, single x pass -----
                wq_sb = wp.tile([P, IT, 512], bf16, tag="w", name="wq_sb")
                wk_sb = wp.tile([P, IT, 512], bf16, tag="w", name="wk_sb")
                wv_sb = wp.tile([P, IT, 512], bf16, tag="w", name="wv_sb")

                q_sb = qkv.tile([P, H, s], bf16, tag="qkv", name="q_sb")
                k_sb = qkv.tile([P, H, s], bf16, tag="qkv", name="k_sb")
                v_sb = qkv.tile([P, NT, 512], bf16, tag="qkv", name="v_sb")

                xblks = []
                for b in range(NB):
                    xblks.append(xp.tile([P, IT, 512], bf16, tag="x",
                                         name=f"xb{b}"))

                # first x block + trig first so PE starts ASAP, then weights
                # interleaved by i-tile (consumption order: q, k, then v)
                def load_xblk(b):
                    blk = slice(512 * b, 512 * (b + 1))
                    for i0 in range(0, IT, 4):
                        nc.sync.dma_start(
                            xblks[b][:, i0:i0 + 4, :],
                            xT[P * i0:P * (i0 + 4), blk]
                            .rearrange("(i p) n -> p i n", p=P))

                load_xblk(0)
                for w_sb, w_dr in ((wq_sb, wqT), (wk_sb, wkT), (wv_sb, wvT)):
                    for i0 in range(0, IT, 4):
                        nc.sync.dma_start(
                            w_sb[:, i0:i0 + 4, :],
                            w_dr[P * i0:P * (i0 + 4), :]
                            .rearrange("(i p) n -> p i n", p=P))

                def proj_block(b):
                    blk = slice(512 * b, 512 * (b + 1))
                    xblk = xblks[b]
                    if b + 1 < NB:
                        load_xblk(b + 1)  # prefetch next block
                    ct = sm.tile([P, 512], f32, tag="ct", name=f"ct{b}", bufs=2)
                    st = sm.tile([P, 512], f32, tag="st", name=f"st{b}", bufs=2)
                    nc.sync.dma_start(ct[:], cosF[:, blk])
                    nc.sync.dma_start(st[:], sinF[:, blk])

                    # q and k projections with fused RoPE
                    for w_sb, dst in ((wq_sb, q_sb), (wk_sb, k_sb)):
                        for ot in range(H):
                            ps = psmm.tile([P, 512], f32, tag="psmm",
                                           name=f"psqk{b}{ot}")
                            for i in range(IT):
                                nc.tensor.matmul(
                                    ps[:],
                                    w_sb[:, i, P * ot:P * (ot + 1)],
                                    xblk[:, i, :],
                                    start=(i == 0), stop=(i == IT - 1))
                            raw = sm.tile([P, 512], f32, tag="raw",
                                          name=f"raw{b}{ot}")
                            nc.scalar.copy(raw[:], ps[:])
                            sw = sm.tile([P, 512], f32, tag="sw", name=f"sw{b}{ot}")
                            nc.sync.dma_start(sw[0:64, :], raw[64:128, :])
                            nc.sync.dma_start(sw[64:128, :], raw[0:64, :])
                            nc.vector.tensor_mul(sw[:], sw[:], st[:])
                            nc.vector.tensor_mul(raw[:], raw[:], ct[:])
                            nc.vector.tensor_add(dst[:, ot, blk], raw[:], sw[:])

                    # v projection: [s-tile, oc]
                    for tt in range(4):
                        ps = psmm.tile([P, 512], f32, tag="psmm", name=f"psv{b}{tt}")
                        for i in range(IT):
                            nc.tensor.matmul(
                                ps[:],
                                xblk[:, i, P * tt:P * (tt + 1)],
                                wv_sb[:, i, :],
                                start=(i == 0), stop=(i == IT - 1))
                        nc.scalar.copy(v_sb[:, 4 * b + tt, :], ps[:])

                # ------------ Phase 2+3: attention / AG / outproj pipeline --
                attn_sb = qkv.tile([P, H, s], bf16, tag="qkv", name="attn_sb")

                def attn_block(j):
                    n_sl = slice(512 * j, 512 * (j + 1))
                    n_mt = 4 * (j + 1)
                    for h in range(H):
                        ps_at = psat.tile([P, 512], f32, tag="psat",
                                          name=f"psat{h}{j}")
                        acc = sm.tile([P, 512], f32r, tag="acc", name=f"acc{h}{j}",
                                      bufs=3)
                        for t in range(n_mt):
                            # diagonal tiles: only columns nn >= off are valid
                            off = max(0, P * (t - 4 * j))
                            w = 512 - off
                            diag = t >= 4 * j
                            ps_s = pssc.tile([P, 512], f32, tag="pssc",
                                             name=f"pss{h}{j}{t}")
                            nc.tensor.matmul(
                                ps_s[:, :w], k_sb[:, h, P * t:P * (t + 1)],
                                q_sb[:, h, 512 * j + off:512 * (j + 1)],
                                start=True, stop=not diag)
                            if diag:
                                nc.tensor.matmul(
                                    ps_s[:, 0:P], negI[:], tric[:],
                                    start=False, stop=True)
                            ex = sm.tile([P, 512], bf16, tag="ex",
                                         name=f"ex{h}{j}{t}", bufs=6)
                            nc.scalar.activation(ex[:, :w], ps_s[:, :w],
                                                 mybir.ActivationFunctionType.Exp)
                            nc.tensor.matmul(
                                ps_at[:, off:512],
                                v_sb[:, t, P * h:P * (h + 1)], ex[:, :w],
                                start=(t == 0), stop=(t == n_mt - 1))
                            if t == 0:
                                nc.vector.tensor_copy(acc[:], ex[:])
                            else:
                                nc.vector.tensor_add(acc[:, off:512],
                                                     acc[:, off:512], ex[:, :w])
                        ps_D = psd.tile([P, 512], f32, tag="psd", name=f"psd{h}{j}")
                        nc.tensor.matmul(ps_D[:], ones[:], acc[:],
                                         start=True, stop=True)
                        recb = sm.tile([P, 512], f32, tag="recb",
                                       name=f"recb{h}{j}")
                        nc.vector.reciprocal(recb[:], ps_D[:])
                        nc.vector.tensor_mul(attn_sb[:, h, n_sl], ps_at[:],
                                             recb[:])
                    # ship this n-block; collective runs on gpsimd/TOPSP only
                    nc.sync.dma_start(
                        attn_part[j][:, :].rearrange("(h p) n -> p h n", p=P),
                        attn_sb[:, :, n_sl])
                    nc.gpsimd.collective_compute(
                        "AllGather",
                        mybir.AluOpType.bypass,
                        replica_groups=[[0, 1, 2, 3], [4, 5, 6, 7]],
                        ins=[attn_part[j][:].opt()],
                        outs=[attn_full[j][:].opt()],
                    )

                def outproj_block(j):
                    for ss in range(4 * j, 4 * (j + 1)):
                        af = sm.tile([P, IT, P], bf16, tag="af", name=f"af{ss}",
                                     bufs=3)
                        nc.gpsimd.dma_start(
                            af[:, :, :],
                            attn_full[j][:, P * (ss % 4):P * (ss % 4 + 1)]
                            .rearrange("(i p) c -> p i c", p=P))
                        ps_o = psmm.tile([P, 512], f32, tag="psmm",
                                         name=f"pso{ss}")
                        for jj in range(IT):
                            nc.tensor.matmul(ps_o[:], af[:, jj, :],
                                             wo_sb[:, jj, :],
                                             start=(jj == 0), stop=(jj == IT - 1))
                        ob = sm.tile([P, 512], f32, tag="ob", name=f"ob{ss}")
                        nc.scalar.copy(ob[:], ps_o[:])
                        nc.sync.dma_start(out[P * ss:P * (ss + 1), :], ob[:])

                # pipeline: project block b then run its attention (which
                # only needs k/v blocks <= b); AGs issue early and hide.
                # outproj afterwards -- PE never queues behind an AllGather.
                for b in range(NB):
                    proj_block(b)
                    attn_block(b)
                wo_sb = wp.tile([P, IT, 512], bf16, tag="w", name="wo_sb")
                for i0 in range(0, IT, 4):
                    nc.sync.dma_start(
                        wo_sb[:, i0:i0 + 4, :],
                        woT[P * i0:P * (i0 + 4), :]
                        .rearrange("(i p) n -> p i n", p=P))
                for j in range(NB):
                    outproj_block(j)

    nc.compile()
    return nc


def _get_nc(s=S, reps=1):
    key = (s, reps)
    if key not in _CACHE:
        _CACHE[key] = _build_nc(s, reps)
    return _CACHE[key]


def _host_prep(x, wq, wk, wv, wo, token_positions, s=S):
    """Build per-core input maps."""
    import ml_dtypes
    bf = ml_dtypes.bfloat16

    # even-first permutation within each head (applied to wq, wk output rows)
    perm = np.concatenate([np.arange(0, D_K, 2), np.arange(1, D_K, 2)])
    full_perm = np.concatenate([h * D_K + perm for h in range(N_HEADS)])
    wq_p = (wq / math.sqrt(D_K))[full_perm, :]
    wk_p = wk[full_perm, :]

    # rope tables per batch: cosF/sinF [128, s]
    power = np.arange(0, D_K, 2, dtype=np.float32) / D_K
    freq = 1.0 / (THETA ** power)                      # [64]
    trig = []
    for b in range(B):
        ang = token_positions[b].astype(np.float32)[None, :] * freq[:, None]  # [64, s]
        c = np.cos(ang).astype(np.float32)
        sn = np.sin(ang).astype(np.float32)
        cosFv = np.concatenate([c, c], axis=0)          # [128, s]
        sinFv = np.concatenate([-sn, sn], axis=0)       # [128, s]
        trig.append((np.ascontiguousarray(cosFv), np.ascontiguousarray(sinFv)))

    in_maps = []
    for c in range(N_CORES):
        b, g = c // 4, c % 4
        rows = slice(OC * g, OC * (g + 1))
        in_maps.append({
            "xT": np.ascontiguousarray(x[b].T).astype(bf),
            "wqT": np.ascontiguousarray(wq_p[rows, :].T).astype(bf),
            "wkT": np.ascontiguousarray(wk_p[rows, :].T).astype(bf),
            "wvT": np.ascontiguousarray(wv[rows, :].T).astype(bf),
            "woT": np.ascontiguousarray(wo[rows, :].T).astype(bf),
            "cosF": trig[b][0],
            "sinF": trig[b][1],
        })
    return in_maps


def kernel(x, wq, wk, wv, wo, token_positions):
    from concourse.bass_utils import run_bass_kernel_spmd

    x = np.asarray(x, dtype=np.float32)
    wq = np.asarray(wq, dtype=np.float32)
    wk = np.asarray(wk, dtype=np.float32)
    wv = np.asarray(wv, dtype=np.float32)
    wo = np.asarray(wo, dtype=np.float32)
    token_positions = np.asarray(token_positions)

    s = x.shape[1]
    nc = _get_nc(s)
    in_maps = _host_prep(x, wq, wk, wv, wo, token_positions, s)
    res = run_bass_kernel_spmd(nc, in_maps, core_ids=list(range(N_CORES)))

    out = np.empty((B, s, D_MODEL), dtype=np.float32)
    for c in range(N_CORES):
        b, g = c // 4, c % 4
        out[b, :, OC * g:OC * (g + 1)] = res.results[c]["out"]
    return out
